# revision 12
# baseline (speedup 1.0000x reference)
"""Trainium2 kernel for nn_LowRank (sparse_attention).

Strategy: data-parallel over batch B=128 across 8 NeuronCores (16 rows each).
The two dominant Linear projections (key/value2: 2 x 137 GMAC, ~95% of FLOPs)
run on-device as tiled matmuls in "Layout B" (rows on partitions, features on
the free dim), immediately followed by a fused CELU + GroupNorm epilogue on
the Scalar/Vector engines (which hide entirely under the TensorE-bound GEMM).
The device returns normalized k / v2; the cheap SCAttention tail runs on host.

CELU identity used on device (GroupNorm is scale-invariant, so celu/alpha is
as good as celu):  celu(x)/a = min(exp(x/a) - 1, relu(x/a)).
"""

import sys

for p in ("/opt/trn_rl_repo",):
    if p not in sys.path:
        sys.path.insert(0, p)

import numpy as np
import ml_dtypes

import concourse.bass as bass
import concourse.bacc as bacc
import concourse.mybir as mybir
from concourse import tile
from concourse.bass_utils import run_bass_kernel_spmd

BF16 = ml_dtypes.bfloat16
F8 = ml_dtypes.float8_e4m3

B, M, E, H, MEM = 128, 1024, 1024, 8, 40
D = E // H
MID = 64
ALPHA = 1.3
EPS = 1e-5
NC = 8
BPC = B // NC          # 16 batch rows per core
R = BPC * M            # 16384 rows of x per core
KC = E // 128          # 8 contraction chunks
RG = 8                 # row-blocks (of 128 rows) per DMA group
NG = R // (128 * RG)   # 16 groups
W_SCALE = 64.0         # fp8 weight pre-scale (keeps W out of subnormals)
FP8_K = False          # k-side projection in fp8 DoubleRow
FP8_V = False          # v2-side projection in fp8 DoubleRow

_CACHE = {}
TRACE = False          # test.py sets True to capture an NTFF profile
TRACE_DIR = None
VERBOSE = False


def _tick(msg, t0):
    import time
    t = time.time()
    if VERBOSE:
        print(f"[kernel] {msg}: {t - t0:.2f}s", flush=True)
    return t


def _build_nc(fp8_k=False, fp8_v=False, rows=R):
    AF = mybir.ActivationFunctionType
    OP = mybir.AluOpType
    nc = bacc.Bacc(trn_type="TRN2")
    specs = []
    for name, fp8 in (("k", fp8_k), ("v", fp8_v)):
        dt_x = mybir.dt.float8e4 if fp8 else mybir.dt.bfloat16
        x_d = nc.dram_tensor(f"x{name}", (E, rows), dt_x, kind="ExternalInput")
        w_d = nc.dram_tensor(f"w{name}", (E, E), dt_x, kind="ExternalInput")
        y_d = nc.dram_tensor(f"y{name}", (rows, E), mybir.dt.bfloat16,
                             kind="ExternalOutput")
        specs.append((name, fp8, dt_x, x_d, w_d, y_d))

    ng = rows // (128 * RG)

    with tile.TileContext(nc) as tc:
        with (
            tc.tile_pool(name="wpool", bufs=1) as wpool,
            tc.tile_pool(name="xpool", bufs=3) as xpool,
            tc.tile_pool(name="opool", bufs=3) as opool,
            tc.tile_pool(name="cpool", bufs=4) as cpool,
            tc.tile_pool(name="spool", bufs=8) as spool,
            tc.tile_pool(name="ppool", bufs=6, space="PSUM") as ppool,
        ):
            eps_t = wpool.tile([128, 1], mybir.dt.float32, tag="eps",
                               name="eps")
            nc.vector.memset(eps_t[:, :], EPS)
            for name, fp8, dt_x, x_d, w_d, y_d in specs:
                # resident transposed weight [128, kc, E]
                wt = wpool.tile([128, KC, E], dt_x, tag=f"w{name}",
                                name=f"wt_{name}")
                nc.sync.dma_start(
                    wt[:, :, :],
                    w_d[:, :].rearrange("(kc p) o -> p kc o", p=128))
                # activation scale: x/alpha, plus undo fp8 weight pre-scale
                sc = 1.0 / (ALPHA * (W_SCALE if fp8 else 1.0))
                for g in range(ng):
                    xt = xpool.tile([128, KC, RG * 128], dt_x, tag="xt",
                                    name="xt")
                    nc.sync.dma_start(
                        xt[:, :, :],
                        x_d[:, :].rearrange("(kc p) r -> p kc r", p=128)
                           [:, :, g * RG * 128:(g + 1) * RG * 128])
                    ob = opool.tile([128, RG, E], mybir.dt.bfloat16, tag="ob",
                                    name="ob")
                    for rb in range(RG):
                        for half in range(2):
                            ps = ppool.tile([128, 512], mybir.dt.float32,
                                            tag="ps", name="ps")
                            if fp8:
                                for kp in range(KC // 2):
                                    nc.tensor.matmul(
                                        ps[:, :],
                                        xt[:, 2 * kp:2 * kp + 2,
                                           rb * 128:(rb + 1) * 128],
                                        wt[:, 2 * kp:2 * kp + 2,
                                           half * 512:(half + 1) * 512],
                                        start=(kp == 0), stop=(kp == KC // 2 - 1),
                                        perf_mode=mybir.MatmulPerfMode.DoubleRow)
                            else:
                                for kc in range(KC):
                                    nc.tensor.matmul(
                                        ps[:, :],
                                        xt[:, kc, rb * 128:(rb + 1) * 128],
                                        wt[:, kc, half * 512:(half + 1) * 512],
                                        start=(kc == 0), stop=(kc == KC - 1))
                            # fused CELU (scale-free) + GroupNorm epilogue
                            t_ = cpool.tile([128, 512], mybir.dt.bfloat16,
                                            tag="t", name="t")
                            r_ = cpool.tile([128, 512], mybir.dt.bfloat16,
                                            tag="r", name="r")
                            nc.scalar.activation(t_[:, :], ps[:, :], AF.Exp,
                                                 bias=0.0, scale=sc)
                            nc.scalar.activation(r_[:, :], ps[:, :], AF.Relu,
                                                 bias=0.0, scale=sc)
                            # w = min(exp(s)-1, relu(s)) = celu(s*a)/a
                            nc.vector.scalar_tensor_tensor(
                                r_[:, :], t_[:, :], 1.0, r_[:, :],
                                OP.subtract, OP.min)
                            st6 = spool.tile([128, 4, 6], mybir.dt.float32,
                                             tag="st6", name="st6")
                            st2 = spool.tile([128, 4, 2], mybir.dt.float32,
                                             tag="st2", name="st2")
                            for gg in range(4):
                                nc.vector.bn_stats(
                                    st6[:, gg, :],
                                    r_[:, gg * 128:(gg + 1) * 128])
                                nc.vector.bn_aggr(st2[:, gg, :], st6[:, gg, :])
                            sd = spool.tile([128, 4], mybir.dt.float32,
                                            tag="sd", name="sd")
                            nc.scalar.activation(sd[:, :], st2[:, :, 1],
                                                 AF.Sqrt, bias=eps_t[:, :])
                            ri = spool.tile([128, 4], mybir.dt.float32,
                                            tag="ri", name="ri")
                            nc.vector.reciprocal(ri[:, :], sd[:, :])
                            for gg in range(4):
                                nc.vector.tensor_scalar(
                                    ob[:, rb,
                                       half * 512 + gg * 128:
                                       half * 512 + (gg + 1) * 128],
                                    r_[:, gg * 128:(gg + 1) * 128],
                                    st2[:, gg, 0:1], ri[:, gg:gg + 1],
                                    OP.subtract, OP.mult)
                    nc.sync.dma_start(
                        y_d[:, :].rearrange("(rb p) e -> p rb e", p=128)
                           [:, g * RG:(g + 1) * RG, :],
                        ob[:, :, :])
    nc.finalize()
    return nc


def _proj_host(x, W, b, g, s):
    # Linear(+bias) + CELU + GroupNorm, rows of x on axis 0.  fp32 host math.
    h = x @ W.T.astype(x.dtype) + b
    h = np.maximum(h, 0.0) + np.minimum(
        0.0, ALPHA * np.expm1(np.minimum(h, 0.0) / ALPHA))
    n = h.shape[0]
    hg = h.reshape(n, H, D)
    mu = hg.mean(-1, keepdims=True)
    var = hg.var(-1, keepdims=True)
    hg = (hg - mu) / np.sqrt(var + EPS)
    return hg.reshape(n, E) * g + s


def kernel(query, key, mask, value1, value2,
           Wq, bq, gq, sq, Wk, bk, gk, sk,
           Wv1, bv1, gv1, sv1, Wv2, bv2, gv2, sv2,
           mem, Wb, bb, Wl, bl, Wl2, bl2):
    import time
    t0 = time.time()
    query = np.asarray(query, np.float32)
    key = np.asarray(key, np.float32)
    value2 = np.asarray(value2, np.float32)
    bk = np.asarray(bk, np.float32)
    bv2 = np.asarray(bv2, np.float32)

    # the device path computes GN without the affine (gamma/beta) term and
    # assumes zero projection bias; gamma/beta are applied exactly via host
    # algebra below.  Non-zero bias would need the slow host fallback.
    use_device = not (np.any(bk) or np.any(bv2))

    ck = ("nc", FP8_K, FP8_V)
    if use_device:
        if ck not in _CACHE:
            _CACHE[ck] = _build_nc(FP8_K, FP8_V)
        nc = _CACHE[ck]
        t0 = _tick("build_nc", t0)

        def prep_w(W, fp8):
            wt = np.ascontiguousarray(np.asarray(W, np.float32).T)
            return (wt * W_SCALE).astype(F8) if fp8 else wt.astype(BF16)

        wk_t = prep_w(Wk, FP8_K)
        wv_t = prep_w(Wv2, FP8_V)
        in_maps = []
        for c in range(NC):
            ks = key[c * BPC:(c + 1) * BPC].reshape(R, E)
            vs = value2[c * BPC:(c + 1) * BPC].reshape(R, E)
            in_maps.append({
                "xk": np.ascontiguousarray(ks.T).astype(F8 if FP8_K else BF16),
                "xv": np.ascontiguousarray(vs.T).astype(F8 if FP8_V else BF16),
                "wk": wk_t,
                "wv": wv_t,
            })
        t0 = _tick("in_maps prep", t0)

        res = run_bass_kernel_spmd(nc, in_maps, core_ids=list(range(NC)),
                                   trace=TRACE, tmpdir=TRACE_DIR)
        _CACHE["last_res"] = res
        results = res.results
        t0 = _tick("device run", t0)

        # normalized (no-affine) projections, [R, E] per core
        k = np.concatenate([r["yk"] for r in results], 0).astype(np.float32)
        v2 = np.concatenate([r["yv"] for r in results], 0).astype(np.float32)
        t0 = _tick("gather", t0)
    else:  # pragma: no cover - generality fallback
        k = _proj_host(key.reshape(-1, E), Wk, bk, np.float32(1), np.float32(0))
        v2 = _proj_host(value2.reshape(-1, E), Wv2, bv2,
                        np.float32(1), np.float32(0))

    k = k.reshape(B, M, H, D).transpose(0, 2, 1, 3)            # [B,H,M,D]
    v2 = v2.reshape(B, M, H, D).transpose(0, 2, 1, 3)

    q = _proj_host(query, Wq, bq, gq, sq).reshape(B, H, D)
    v1 = _proj_host(value1, Wv1, bv1, gv1, sv1).reshape(B, H, D)

    mem_hd = np.broadcast_to(mem, (B, MEM, E)).reshape(
        B, MEM, H, D).transpose(0, 2, 1, 3)
    sqD = np.float32(np.sqrt(np.float32(D)))
    sqM = np.float32(np.sqrt(np.float32(MEM)))
    mask_full = np.concatenate([mask, mask[:, :MEM]], axis=-1).astype(np.float32)
    Mt = M + MEM

    gk_h = np.asarray(gk, np.float32).reshape(H, D)
    sk_h = np.asarray(sk, np.float32).reshape(H, D)
    gv2_h = np.asarray(gv2, np.float32).reshape(H, D)
    sv2_h = np.asarray(sv2, np.float32).reshape(H, D)

    # ---- SCAttention on host ----
    # attn_map for the projected part: (k*gk + sk) * q = k*(q*gk) + q*sk.
    qg = q * gk_h[None]                                        # [B,H,D]
    qs_const = q * sk_h[None]                                  # [B,H,D] additive
    # h = relu(attn_map @ Wb.T + bb): projected part gets a constant per (b,h)
    hc = qs_const @ Wb.T                                       # [B,H,MID]
    am_proj = qg[:, :, None, :] * k                            # [B,H,M,D]
    h1 = am_proj.reshape(-1, D) @ Wb.T
    h1 = h1.reshape(B, H, M, MID) + hc[:, :, None, :] + bb
    am_mem = q[:, :, None, :] * (sqD * mem_hd)                 # [B,H,MEM,D]
    h2 = am_mem.reshape(-1, D) @ Wb.T + bb
    h2 = h2.reshape(B, H, MEM, MID)
    h = np.concatenate([h1, h2], axis=2)                       # [B,H,Mt,MID]
    np.maximum(h, 0.0, out=h)
    t0 = _tick("attn_map+h", t0)

    mext = mask_full[:, None, :, None]
    pool = (h * mext).sum(axis=2) / mext.sum(axis=2)           # [B,H,MID]
    alpha_sp = h.reshape(-1, MID) @ Wl[0] + bl[0]
    alpha_sp = alpha_sp.reshape(B, H, Mt)
    alpha_sp = np.where(mask_full[:, None, :] == 0, np.float32(-1e9), alpha_sp)
    alpha_sp = alpha_sp - alpha_sp.max(-1, keepdims=True)
    np.exp(alpha_sp, out=alpha_sp)
    alpha_sp /= alpha_sp.sum(-1, keepdims=True)
    alpha_ch = 1.0 / (1.0 + np.exp(-(pool @ Wl2.T + bl2)))     # [B,H,D]
    # v2p: projected part Sum_m a*(v2*gv2+sv2) = gv2*(Sum a v2) + sv2*Sum(a)
    a1 = alpha_sp[:, :, :M]
    a2 = alpha_sp[:, :, M:]
    v2p = np.einsum("bhm,bhmd->bhd", a1, v2, optimize=True)
    v2p = v2p * gv2_h[None] + sv2_h[None] * a1.sum(-1)[:, :, None]
    v2p += np.einsum("bhm,bhmd->bhd", a2, sqM * mem_hd, optimize=True)
    attn = v1 * v2p * alpha_ch
    _tick("rest of epilogue", t0)
    return attn.reshape(B, E).astype(np.float32)


# revision 17
# speedup vs baseline: 2.1517x; 2.1517x over previous
"""Trainium2 kernel for nn_LowRank (sparse_attention).

Strategy: data-parallel over batch B=128 across 8 NeuronCores (16 rows each).
The two dominant Linear projections (key/value2: 2 x 137 GMAC, ~95% of FLOPs)
run on-device:
  - k-side in fp8-e4m3 with DoubleRow perf mode (2 MACs/cell/cycle).  A
    numerical simulation shows k-side fp8 is safe (2.2e-3 end-to-end vs
    2.1e-3 for bf16): k-path errors are damped by the near-uniform softmax
    and the sigmoid channel gate.  Weights are pre-scaled by 64 to stay out
    of fp8 subnormals; the PSUM->SBUF copy divides it back out.
  - v2-side in bf16 (fp8 here fails the tolerance: v2 feeds the output
    directly).
The cheap epilogue (CELU, GroupNorm, SCAttention) runs on host in fp32.
"""

import sys

for p in ("/opt/trn_rl_repo",):
    if p not in sys.path:
        sys.path.insert(0, p)

import numpy as np
import ml_dtypes

import concourse.bass as bass
import concourse.bacc as bacc
import concourse.mybir as mybir
from concourse import tile
from concourse.bass_utils import run_bass_kernel_spmd

BF16 = ml_dtypes.bfloat16
F8 = ml_dtypes.float8_e4m3

B, M, E, H, MEM = 128, 1024, 1024, 8, 40
D = E // H
MID = 64
ALPHA = 1.3
EPS = 1e-5
NC = 8
BPC = B // NC          # 16 batch rows per core
R = BPC * M            # 16384 rows of x per core
KC = E // 128          # 8 contraction chunks
W_SCALE = 64.0         # fp8 weight pre-scale (keeps W out of subnormals)
FP8_K = True           # k-side projection in fp8 DoubleRow

# k-side (Layout A: features on partitions, weight-stationary, fp8 DR)
GA = 4                 # psum tiles per stationary weight load
NBLK = 512             # columns (rows of x) per psum tile
NGA = R // (NBLK * GA) # 8 column groups

# v2-side (Layout B: rows on partitions, bf16)
RG = 8                 # row-blocks of 128 per DMA group
NGB = R // (128 * RG)  # 16 groups

_CACHE = {}
TRACE = False          # test.py sets True to capture an NTFF profile
TRACE_DIR = None
VERBOSE = False


def _tick(msg, t0):
    import time
    t = time.time()
    if VERBOSE:
        print(f"[kernel] {msg}: {t - t0:.2f}s", flush=True)
    return t


def _build_nc(fp8_k=FP8_K):
    OP = mybir.AluOpType
    nc = bacc.Bacc(trn_type="TRN2")
    dt_k = mybir.dt.float8e4 if fp8_k else mybir.dt.bfloat16
    xk = nc.dram_tensor("xk", (E, R), dt_k, kind="ExternalInput")
    wk = nc.dram_tensor("wk", (E, E), dt_k, kind="ExternalInput")
    yk = nc.dram_tensor("yk", (E, R), mybir.dt.bfloat16, kind="ExternalOutput")
    xv = nc.dram_tensor("xv", (E, R), mybir.dt.bfloat16, kind="ExternalInput")
    wv = nc.dram_tensor("wv", (E, E), mybir.dt.bfloat16, kind="ExternalInput")
    yv = nc.dram_tensor("yv", (R, E), mybir.dt.bfloat16, kind="ExternalOutput")

    with tile.TileContext(nc) as tc:
        with (
            tc.tile_pool(name="wpool", bufs=1) as wpool,
            tc.tile_pool(name="xpool", bufs=3) as xpool,
            tc.tile_pool(name="opool", bufs=2) as opool,
            tc.tile_pool(name="ppool", bufs=8, space="PSUM") as ppool,
        ):
            # ---- k-side: Layout A, weight-stationary, fp8 DoubleRow ----
            # out[feat, rows]; stationary lhsT = W^T chunk, moving rhs = x^T
            wkt = wpool.tile([128, KC, E], dt_k, tag="wk", name="wkt")
            nc.sync.dma_start(
                wkt[:, :, :], wk[:, :].rearrange("(kc p) o -> p kc o", p=128))
            kinv = 1.0 / (W_SCALE if fp8_k else 1.0)
            CW = NBLK * GA     # 2048 columns per group
            for g in range(NGA):
                xt = xpool.tile([128, KC, CW], dt_k, tag="xt", name="xkt")
                nc.sync.dma_start(
                    xt[:, :, :],
                    xk[:, :].rearrange("(kc p) r -> p kc r", p=128)
                      [:, :, g * CW:(g + 1) * CW])
                ob = opool.tile([128, KC, CW], mybir.dt.bfloat16, tag="ot",
                                name="okt")
                for jc in range(KC):
                    pss = [ppool.tile([128, NBLK], mybir.dt.float32,
                                      tag="ps", name=f"psk{i}")
                           for i in range(GA)]
                    if fp8_k:
                        for kp in range(KC // 2):
                            lhs = wkt[:, 2 * kp:2 * kp + 2,
                                      jc * 128:(jc + 1) * 128]
                            for i in range(GA):
                                nc.tensor.matmul(
                                    pss[i][:, :], lhs,
                                    xt[:, 2 * kp:2 * kp + 2,
                                       i * NBLK:(i + 1) * NBLK],
                                    start=(kp == 0), stop=(kp == KC // 2 - 1),
                                    perf_mode=mybir.MatmulPerfMode.DoubleRow)
                    else:
                        for kc in range(KC):
                            lhs = wkt[:, kc, jc * 128:(jc + 1) * 128]
                            for i in range(GA):
                                nc.tensor.matmul(
                                    pss[i][:, :], lhs,
                                    xt[:, kc, i * NBLK:(i + 1) * NBLK],
                                    start=(kc == 0), stop=(kc == KC - 1))
                    for i in range(GA):
                        nc.vector.tensor_scalar(
                            ob[:, jc, i * NBLK:(i + 1) * NBLK],
                            pss[i][:, :], kinv, None, OP.mult)
                nc.sync.dma_start(
                    yk[:, :].rearrange("(jc p) r -> p jc r", p=128)
                      [:, :, g * CW:(g + 1) * CW],
                    ob[:, :, :])

            # ---- v2-side: Layout B, x-stationary, bf16 ----
            wvt = wpool.tile([128, KC, E], mybir.dt.bfloat16, tag="wv",
                             name="wvt")
            nc.sync.dma_start(
                wvt[:, :, :], wv[:, :].rearrange("(kc p) o -> p kc o", p=128))
            for g in range(NGB):
                xt = xpool.tile([128, KC, RG * 128], mybir.dt.bfloat16,
                                tag="xt", name="xvt")
                nc.sync.dma_start(
                    xt[:, :, :],
                    xv[:, :].rearrange("(kc p) r -> p kc r", p=128)
                      [:, :, g * RG * 128:(g + 1) * RG * 128])
                ob = opool.tile([128, RG, E], mybir.dt.bfloat16, tag="ot",
                                name="ovt")
                for rb in range(RG):
                    for half in range(2):
                        ps = ppool.tile([128, 512], mybir.dt.float32,
                                        tag="ps", name="psv")
                        for kc in range(KC):
                            nc.tensor.matmul(
                                ps[:, :],
                                xt[:, kc, rb * 128:(rb + 1) * 128],
                                wvt[:, kc, half * 512:(half + 1) * 512],
                                start=(kc == 0), stop=(kc == KC - 1))
                        nc.vector.tensor_copy(
                            ob[:, rb, half * 512:(half + 1) * 512], ps[:, :])
                nc.sync.dma_start(
                    yv[:, :].rearrange("(rb p) e -> p rb e", p=128)
                      [:, g * RG:(g + 1) * RG, :],
                    ob[:, :, :])
    nc.finalize()
    return nc


def _celu_gn_rows(y, b_, g, s):
    # y: [N, E] fp32 pre-activation rows; CELU + GroupNorm(H groups).
    y = y + b_ if np.any(b_) else y
    y = np.maximum(y, 0.0) + np.minimum(
        0.0, ALPHA * np.expm1(np.minimum(y, 0.0) / ALPHA))
    n = y.shape[0]
    yg = y.reshape(n, H, D)
    mu = yg.mean(-1, keepdims=True)
    var = yg.var(-1, keepdims=True)
    yg -= mu
    yg /= np.sqrt(var + EPS)
    return yg.reshape(n, E) * g + s


def _proj_host(x, W, b, g, s):
    return _celu_gn_rows(x @ W.T.astype(x.dtype), b, g, s)


def kernel(query, key, mask, value1, value2,
           Wq, bq, gq, sq, Wk, bk, gk, sk,
           Wv1, bv1, gv1, sv1, Wv2, bv2, gv2, sv2,
           mem, Wb, bb, Wl, bl, Wl2, bl2):
    import time
    t0 = time.time()
    query = np.asarray(query, np.float32)
    key = np.asarray(key, np.float32)
    value2 = np.asarray(value2, np.float32)

    ckey = ("nc", FP8_K)
    if ckey not in _CACHE:
        _CACHE[ckey] = _build_nc(FP8_K)
    nc = _CACHE[ckey]
    t0 = _tick("build_nc", t0)

    wk_t = np.ascontiguousarray(np.asarray(Wk, np.float32).T)
    wk_t = (wk_t * W_SCALE).astype(F8) if FP8_K else wk_t.astype(BF16)
    wv_t = np.ascontiguousarray(
        np.asarray(Wv2, np.float32).T).astype(BF16)
    in_maps = []
    for c in range(NC):
        ks = key[c * BPC:(c + 1) * BPC].reshape(R, E)
        vs = value2[c * BPC:(c + 1) * BPC].reshape(R, E)
        in_maps.append({
            "xk": np.ascontiguousarray(ks.T).astype(F8 if FP8_K else BF16),
            "xv": np.ascontiguousarray(vs.T).astype(BF16),
            "wk": wk_t,
            "wv": wv_t,
        })
    t0 = _tick("in_maps prep", t0)

    res = run_bass_kernel_spmd(nc, in_maps, core_ids=list(range(NC)),
                               trace=TRACE, tmpdir=TRACE_DIR)
    _CACHE["last_res"] = res
    results = res.results
    t0 = _tick("device run", t0)

    # k-side comes back transposed [E, R] -> [R, E]
    k = np.concatenate(
        [np.ascontiguousarray(r["yk"].T) for r in results], 0
    ).astype(np.float32)
    v2 = np.concatenate([r["yv"] for r in results], 0).astype(np.float32)
    t0 = _tick("gather", t0)

    k = _celu_gn_rows(k, bk, gk, sk)
    v2 = _celu_gn_rows(v2, bv2, gv2, sv2)
    k = k.reshape(B, M, H, D).transpose(0, 2, 1, 3)            # [B,H,M,D]
    v2 = v2.reshape(B, M, H, D).transpose(0, 2, 1, 3)
    t0 = _tick("celu_gn", t0)

    q = _proj_host(query, Wq, bq, gq, sq).reshape(B, H, D)
    v1 = _proj_host(value1, Wv1, bv1, gv1, sv1).reshape(B, H, D)

    mem_hd = np.broadcast_to(mem, (B, MEM, E)).reshape(
        B, MEM, H, D).transpose(0, 2, 1, 3)
    sqD = np.float32(np.sqrt(np.float32(D)))
    sqM = np.float32(np.sqrt(np.float32(MEM)))
    k = np.concatenate([k, sqD * mem_hd], axis=2)              # [B,H,Mt,D]
    mask_full = np.concatenate([mask, mask[:, :MEM]], axis=-1).astype(np.float32)
    Mt = M + MEM

    attn_map = q[:, :, None, :] * k                            # [B,H,Mt,D]
    h = attn_map.reshape(-1, D) @ Wb.T + bb                    # [BHMt, MID]
    np.maximum(h, 0.0, out=h)
    h = h.reshape(B, H, Mt, MID)
    t0 = _tick("attn_map+h", t0)

    mext = mask_full[:, None, :, None]
    pool = (h * mext).sum(axis=2) / mext.sum(axis=2)           # [B,H,MID]
    alpha_sp = h.reshape(-1, MID) @ Wl[0] + bl[0]
    alpha_sp = alpha_sp.reshape(B, H, Mt)
    alpha_sp = np.where(mask_full[:, None, :] == 0, np.float32(-1e9), alpha_sp)
    alpha_sp = alpha_sp - alpha_sp.max(-1, keepdims=True)
    np.exp(alpha_sp, out=alpha_sp)
    alpha_sp /= alpha_sp.sum(-1, keepdims=True)
    alpha_ch = 1.0 / (1.0 + np.exp(-(pool @ Wl2.T + bl2)))     # [B,H,D]
    v2p = np.einsum("bhm,bhmd->bhd", alpha_sp[:, :, :M], v2, optimize=True)
    v2p += np.einsum("bhm,bhmd->bhd", alpha_sp[:, :, M:], sqM * mem_hd,
                     optimize=True)
    attn = v1 * v2p * alpha_ch
    _tick("rest of epilogue", t0)
    return attn.reshape(B, E).astype(np.float32)


# revision 20
# speedup vs baseline: 2.1558x; 1.0019x over previous
"""Trainium2 kernel for nn_LowRank (sparse_attention).

Strategy: data-parallel over batch B=128 across 8 NeuronCores (16 rows each).
The two dominant Linear projections (key/value2: 2 x 137 GMAC, ~95% of FLOPs)
run on-device:
  - k-side in fp8-e4m3 with DoubleRow perf mode (2 MACs/cell/cycle).  A
    numerical simulation shows k-side fp8 is safe (2.2e-3 end-to-end vs
    2.1e-3 for bf16): k-path errors are damped by the near-uniform softmax
    and the sigmoid channel gate.  Weights are pre-scaled by 64 to stay out
    of fp8 subnormals; the PSUM->SBUF copy divides it back out.
  - v2-side in bf16 (fp8 here fails the tolerance: v2 feeds the output
    directly).
The cheap epilogue (CELU, GroupNorm, SCAttention) runs on host in fp32.
"""

import sys

for p in ("/opt/trn_rl_repo",):
    if p not in sys.path:
        sys.path.insert(0, p)

import numpy as np
import ml_dtypes

import concourse.bass as bass
import concourse.bacc as bacc
import concourse.mybir as mybir
from concourse import tile
from concourse.bass_utils import run_bass_kernel_spmd

BF16 = ml_dtypes.bfloat16
F8 = ml_dtypes.float8_e4m3

B, M, E, H, MEM = 128, 1024, 1024, 8, 40
D = E // H
MID = 64
ALPHA = 1.3
EPS = 1e-5
NC = 8
BPC = B // NC          # 16 batch rows per core
R = BPC * M            # 16384 rows of x per core
KC = E // 128          # 8 contraction chunks
W_SCALE = 64.0         # fp8 weight pre-scale (keeps W out of subnormals)
FP8_K = True           # k-side projection in fp8 DoubleRow

# k-side (Layout A: features on partitions, weight-stationary, fp8 DR)
GA = 4                 # psum tiles per stationary weight load
NBLK = 512             # columns (rows of x) per psum tile
NGA = R // (NBLK * GA) # 8 column groups

# v2-side (Layout B: rows on partitions, bf16)
RG = 8                 # row-blocks of 128 per DMA group
NGB = R // (128 * RG)  # 16 groups

_CACHE = {}
TRACE = False          # test.py sets True to capture an NTFF profile
TRACE_DIR = None
VERBOSE = False


def _tick(msg, t0):
    import time
    t = time.time()
    if VERBOSE:
        print(f"[kernel] {msg}: {t - t0:.2f}s", flush=True)
    return t


def _build_nc(fp8_k=FP8_K):
    OP = mybir.AluOpType
    nc = bacc.Bacc(trn_type="TRN2")
    dt_k = mybir.dt.float8e4 if fp8_k else mybir.dt.bfloat16
    xk = nc.dram_tensor("xk", (E, R), dt_k, kind="ExternalInput")
    wk = nc.dram_tensor("wk", (E, E), dt_k, kind="ExternalInput")
    yk = nc.dram_tensor("yk", (E, R), mybir.dt.bfloat16, kind="ExternalOutput")
    xv = nc.dram_tensor("xv", (E, R), mybir.dt.bfloat16, kind="ExternalInput")
    wv = nc.dram_tensor("wv", (E, E), mybir.dt.bfloat16, kind="ExternalInput")
    yv = nc.dram_tensor("yv", (R, E), mybir.dt.bfloat16, kind="ExternalOutput")

    with tile.TileContext(nc) as tc:
        with (
            tc.tile_pool(name="wpool", bufs=1) as wpool,
            tc.tile_pool(name="xpool", bufs=3) as xpool,
            tc.tile_pool(name="opool", bufs=2) as opool,
            tc.tile_pool(name="ppool", bufs=8, space="PSUM") as ppool,
        ):
            # ---- k-side: Layout A, weight-stationary, fp8 DoubleRow ----
            # out[feat, rows]; stationary lhsT = W^T chunk, moving rhs = x^T
            wkt = wpool.tile([128, KC, E], dt_k, tag="wk", name="wkt")
            nc.sync.dma_start(
                wkt[:, :, :], wk[:, :].rearrange("(kc p) o -> p kc o", p=128))
            kinv = 1.0 / (W_SCALE if fp8_k else 1.0)
            CW = NBLK * GA     # 2048 columns per group
            for g in range(NGA):
                xt = xpool.tile([128, KC, CW], dt_k, tag="xt", name="xkt")
                nc.sync.dma_start(
                    xt[:, :, :],
                    xk[:, :].rearrange("(kc p) r -> p kc r", p=128)
                      [:, :, g * CW:(g + 1) * CW])
                ob = opool.tile([128, KC, CW], mybir.dt.bfloat16, tag="ot",
                                name="okt")
                for jc in range(KC):
                    pss = [ppool.tile([128, NBLK], mybir.dt.float32,
                                      tag="ps", name=f"psk{i}")
                           for i in range(GA)]
                    if fp8_k:
                        for kp in range(KC // 2):
                            lhs = wkt[:, 2 * kp:2 * kp + 2,
                                      jc * 128:(jc + 1) * 128]
                            for i in range(GA):
                                nc.tensor.matmul(
                                    pss[i][:, :], lhs,
                                    xt[:, 2 * kp:2 * kp + 2,
                                       i * NBLK:(i + 1) * NBLK],
                                    start=(kp == 0), stop=(kp == KC // 2 - 1),
                                    perf_mode=mybir.MatmulPerfMode.DoubleRow)
                    else:
                        for kc in range(KC):
                            lhs = wkt[:, kc, jc * 128:(jc + 1) * 128]
                            for i in range(GA):
                                nc.tensor.matmul(
                                    pss[i][:, :], lhs,
                                    xt[:, kc, i * NBLK:(i + 1) * NBLK],
                                    start=(kc == 0), stop=(kc == KC - 1))
                    for i in range(GA):
                        nc.vector.tensor_scalar(
                            ob[:, jc, i * NBLK:(i + 1) * NBLK],
                            pss[i][:, :], kinv, None, OP.mult)
                nc.sync.dma_start(
                    yk[:, :].rearrange("(jc p) r -> p jc r", p=128)
                      [:, :, g * CW:(g + 1) * CW],
                    ob[:, :, :])

            # ---- v2-side: Layout B, x-stationary, bf16 ----
            wvt = wpool.tile([128, KC, E], mybir.dt.bfloat16, tag="wv",
                             name="wvt")
            nc.sync.dma_start(
                wvt[:, :, :], wv[:, :].rearrange("(kc p) o -> p kc o", p=128))
            for g in range(NGB):
                xt = xpool.tile([128, KC, RG * 128], mybir.dt.bfloat16,
                                tag="xt", name="xvt")
                nc.sync.dma_start(
                    xt[:, :, :],
                    xv[:, :].rearrange("(kc p) r -> p kc r", p=128)
                      [:, :, g * RG * 128:(g + 1) * RG * 128])
                ob = opool.tile([128, RG, E], mybir.dt.bfloat16, tag="ot",
                                name="ovt")
                for rb in range(RG):
                    for half in range(2):
                        ps = ppool.tile([128, 512], mybir.dt.float32,
                                        tag="ps", name="psv")
                        for kc in range(KC):
                            nc.tensor.matmul(
                                ps[:, :],
                                xt[:, kc, rb * 128:(rb + 1) * 128],
                                wvt[:, kc, half * 512:(half + 1) * 512],
                                start=(kc == 0), stop=(kc == KC - 1))
                        nc.vector.tensor_copy(
                            ob[:, rb, half * 512:(half + 1) * 512], ps[:, :])
                nc.sync.dma_start(
                    yv[:, :].rearrange("(rb p) e -> p rb e", p=128)
                      [:, g * RG:(g + 1) * RG, :],
                    ob[:, :, :])
    nc.finalize()
    return nc


def _celu_gn_rows(y, b_, g, s, out=None):
    # y: [N, E] fp32 pre-activation rows; CELU + GroupNorm(H groups).
    if np.any(b_):
        y = y + b_
    neg = np.minimum(y, 0.0)
    neg /= ALPHA
    np.expm1(neg, out=neg)
    neg *= ALPHA
    pos = np.maximum(y, 0.0, out=y if out is y else None)
    y = np.minimum(neg, 0.0, out=neg)
    y += pos
    n = y.shape[0]
    yg = y.reshape(n, H, D)
    mu = yg.mean(-1, keepdims=True)
    var = yg.var(-1, keepdims=True)
    yg -= mu
    yg /= np.sqrt(var + EPS)
    y = yg.reshape(n, E)
    if not (np.all(g == 1.0) and np.all(s == 0.0)):
        y *= g
        y += s
    if out is not None and out is not y:
        np.copyto(out, y)
    return y


def _proj_host(x, W, b, g, s):
    return _celu_gn_rows(x @ W.T.astype(x.dtype), b, g, s)


def kernel(query, key, mask, value1, value2,
           Wq, bq, gq, sq, Wk, bk, gk, sk,
           Wv1, bv1, gv1, sv1, Wv2, bv2, gv2, sv2,
           mem, Wb, bb, Wl, bl, Wl2, bl2):
    import time
    t0 = time.time()
    query = np.asarray(query, np.float32)
    key = np.asarray(key, np.float32)
    value2 = np.asarray(value2, np.float32)

    ckey = ("nc", FP8_K)
    if ckey not in _CACHE:
        _CACHE[ckey] = _build_nc(FP8_K)
    nc = _CACHE[ckey]
    t0 = _tick("build_nc", t0)

    wk_t = np.ascontiguousarray(np.asarray(Wk, np.float32).T)
    wk_t = (wk_t * W_SCALE).astype(F8) if FP8_K else wk_t.astype(BF16)
    wv_t = np.ascontiguousarray(
        np.asarray(Wv2, np.float32).T).astype(BF16)
    in_maps = []
    for c in range(NC):
        ks = key[c * BPC:(c + 1) * BPC].reshape(R, E)
        vs = value2[c * BPC:(c + 1) * BPC].reshape(R, E)
        in_maps.append({
            "xk": np.ascontiguousarray(ks.T).astype(F8 if FP8_K else BF16),
            "xv": np.ascontiguousarray(vs.T).astype(BF16),
            "wk": wk_t,
            "wv": wv_t,
        })
    t0 = _tick("in_maps prep", t0)

    res = run_bass_kernel_spmd(nc, in_maps, core_ids=list(range(NC)),
                               trace=TRACE, tmpdir=TRACE_DIR)
    _CACHE["last_res"] = res
    results = res.results
    t0 = _tick("device run", t0)

    # fused gather + CELU + GroupNorm, per-core chunks; layout [B,M,H,D]
    k = np.empty((B, M, E), np.float32)
    v2 = np.empty((B, M, E), np.float32)
    for c, r in enumerate(results):
        yk_c = np.asarray(r["yk"])                             # [E, R] bf16
        kc = k[c * BPC:(c + 1) * BPC].reshape(R, E)
        np.copyto(kc, yk_c.T)
        _celu_gn_rows(kc, bk, gk, sk, out=kc)
        yv_c = np.asarray(r["yv"])                             # [R, E] bf16
        vc = v2[c * BPC:(c + 1) * BPC].reshape(R, E)
        np.copyto(vc, yv_c)
        _celu_gn_rows(vc, bv2, gv2, sv2, out=vc)
    k = k.reshape(B, M, H, D)
    v2 = v2.reshape(B, M, H, D)
    t0 = _tick("gather+celu_gn", t0)

    q = _proj_host(query, Wq, bq, gq, sq).reshape(B, H, D)
    v1 = _proj_host(value1, Wv1, bv1, gv1, sv1).reshape(B, H, D)

    mem_hd = np.broadcast_to(mem, (B, MEM, E)).reshape(B, MEM, H, D)
    sqD = np.float32(np.sqrt(np.float32(D)))
    sqM = np.float32(np.sqrt(np.float32(MEM)))
    k = np.concatenate([k, sqD * mem_hd], axis=1)              # [B,Mt,H,D]
    mask_full = np.concatenate([mask, mask[:, :MEM]], axis=-1).astype(np.float32)
    Mt = M + MEM

    attn_map = q[:, None, :, :] * k                            # [B,Mt,H,D]
    h = attn_map.reshape(-1, D) @ Wb.T + bb                    # [BMtH, MID]
    np.maximum(h, 0.0, out=h)
    h = h.reshape(B, Mt, H, MID)
    t0 = _tick("attn_map+h", t0)

    mext = mask_full[:, :, None, None]
    pool = (h * mext).sum(axis=1) / mext.sum(axis=1)           # [B,H,MID]
    alpha_sp = h.reshape(-1, MID) @ Wl[0] + bl[0]
    alpha_sp = alpha_sp.reshape(B, Mt, H)
    alpha_sp = np.where(mask_full[:, :, None] == 0, np.float32(-1e9), alpha_sp)
    alpha_sp = alpha_sp - alpha_sp.max(1, keepdims=True)
    np.exp(alpha_sp, out=alpha_sp)
    alpha_sp /= alpha_sp.sum(1, keepdims=True)
    alpha_ch = 1.0 / (1.0 + np.exp(-(pool @ Wl2.T + bl2)))     # [B,H,D]
    v2p = np.einsum("bmh,bmhd->bhd", alpha_sp[:, :M], v2, optimize=True)
    v2p += np.einsum("bmh,bmhd->bhd", alpha_sp[:, M:], sqM * mem_hd,
                     optimize=True)
    attn = v1 * v2p * alpha_ch
    _tick("rest of epilogue", t0)
    return attn.reshape(B, E).astype(np.float32)
